# revision 1
# baseline (speedup 1.0000x reference)
"""AttnMPNN (GNN message passing w/ edge softmax) on 8 Trainium2 NeuronCores.

Single-program, dst-sharded design (v2):
  - 8 cores each own NPC = N/8 destination nodes (49 slots x 128).
  - Host prep: edges bucketed by (core, dst slot, src half); gather indices
    padded with TRAILING -1 (dma_gather ucode skips them -> gen cost tracks
    the real edge count). pm (dst mod 128) padded with -1 (sel cols vanish).
    Host also supplies nf in bf16 node-major AND transposed (nfT) so the
    device builds C = [A | nf] with zero PE transposes.
  - Device phase 1: A = nf @ W1 + ba via matmul(lhsT=nfT chunk, rhs=W1),
    written with the nf half as C rows [NF_PAD, 128] bf16 in DRAM;
    B = nf_own @ W2 kept in SBUF [128, SLOTS, 64] bf16.
  - Device phase 2 per slot: 2x dma_gather of C rows (edge-on-partition),
    one bf16 sel + one bf16 selT is_equal build, B-expansion via PE,
    h = relu(A+B), logits, leaky, exp; payload [w*nf | w] (weight folded
    into payload, not sel); 18 accumulation matmuls into PSUM [128, 65];
    finale out = [nf_own | agg]@Wn + bn via two accumulating matmuls
    (lhsT = nfT-own slice / transposed agg), no nf transposes.
  - Softmax max-subtraction dropped (invariant; logits are O(1)).
"""

import numpy as np

P = 128
D = 64


def _ceil_div(a, b):
    return (a + b - 1) // b


def _wrap16(arr2d):
    """[S, n] int16 -> [S, 128, n//16] wrapped by 16, replicated to 8 groups."""
    S, n = arr2d.shape
    w = arr2d.reshape(S, n // 16, 16).transpose(0, 2, 1)
    return np.ascontiguousarray(np.tile(w, (1, 8, 1)))


def _build_program(cfg):
    import concourse.bass as bass
    import concourse.tile as tile
    from concourse import bacc, mybir

    NF_PAD = cfg["NF_PAD"]
    HALF = cfg["HALF"]
    NPAD = cfg["NPAD"]
    SLOTS = NPAD // P
    B0 = cfg["B0"]
    B1 = cfg["B1"]
    BPS = B0 + B1
    OWN0 = cfg["OWN0_PER_CORE"]  # not used on device (same program all cores)
    f32 = mybir.dt.float32
    bf16 = mybir.dt.bfloat16
    i16 = mybir.dt.int16

    _AFT = mybir.ActivationFunctionType
    _ALT = mybir.AxisListType
    _ALU = mybir.AluOpType

    nc = bacc.Bacc("TRN2", target_bir_lowering=False, debug=False,
                   enable_asserts=False)

    u32 = mybir.dt.uint32
    t_nfb = nc.dram_tensor("nfb", (NF_PAD, D), bf16, kind="ExternalInput")
    t_nft = nc.dram_tensor("nft", (D + 1, NF_PAD), bf16, kind="ExternalInput")
    t_nfto = nc.dram_tensor("nfto", (D + 1, NPAD), bf16, kind="ExternalInput")
    t_w1 = nc.dram_tensor("w1", (D + 1, D), bf16, kind="ExternalInput")
    t_w2 = nc.dram_tensor("w2", (D + 1, D), bf16, kind="ExternalInput")
    t_cnt = nc.dram_tensor("cnt", (1, SLOTS * 2), u32, kind="ExternalInput")
    t_wfc = nc.dram_tensor("wfc", (D,), bf16, kind="ExternalInput")
    t_wn1 = nc.dram_tensor("wn1", (D, D), bf16, kind="ExternalInput")
    t_wn2 = nc.dram_tensor("wn2", (D, D), bf16, kind="ExternalInput")
    t_bn = nc.dram_tensor("bn", (D,), f32, kind="ExternalInput")
    t_iob = nc.dram_tensor("iob", (P, P), bf16, kind="ExternalInput")
    t_ioc = nc.dram_tensor("ioc", (P, 1), bf16, kind="ExternalInput")
    t_is0 = nc.dram_tensor("is0", (SLOTS, P, B0 * 8), i16, kind="ExternalInput")
    t_is1 = nc.dram_tensor("is1", (SLOTS, P, B1 * 8), i16, kind="ExternalInput")
    t_pm = nc.dram_tensor("pm", (SLOTS, P, BPS), bf16, kind="ExternalInput")
    t_pmT = nc.dram_tensor("pmT", (SLOTS, BPS * P), bf16, kind="ExternalInput")
    t_dginv = nc.dram_tensor("dginv", (P, SLOTS), f32, kind="ExternalInput")
    t_C = nc.dram_tensor("C_scr", (NF_PAD, 2 * D), bf16,
                         kind="ExternalOutput")
    t_out = nc.dram_tensor("out", (NPAD, D), f32, kind="ExternalOutput")

    with tile.TileContext(nc) as tc:
        import contextlib
        ctx = contextlib.ExitStack()
        with ctx:
            const_p = ctx.enter_context(tc.tile_pool(name="const", bufs=1))

            # ---- persistent constants
            from concourse.masks import make_identity
            ident = const_p.tile([P, P], f32)
            make_identity(nc, ident[:])
            w1_sb = const_p.tile([D + 1, D], bf16)
            nc.sync.dma_start(w1_sb[:], t_w1.ap())
            w2_sb = const_p.tile([D + 1, D], bf16)
            nc.sync.dma_start(w2_sb[:], t_w2.ap())
            wn1_sb = const_p.tile([D, D], bf16)
            nc.sync.dma_start(wn1_sb[:], t_wn1.ap())
            wn2_sb = const_p.tile([D, D], bf16)
            nc.sync.dma_start(wn2_sb[:], t_wn2.ap())
            cnt_sb = const_p.tile([1, SLOTS * 2], u32)
            nc.sync.dma_start(cnt_sb[:], t_cnt.ap())
            bn_b = const_p.tile([P, D], f32)
            nc.sync.dma_start(bn_b[:], t_bn.ap()[None, :].broadcast_to((P, D)))
            wfc_b = const_p.tile([P, D], bf16)
            nc.sync.dma_start(wfc_b[:], t_wfc.ap()[None, :].broadcast_to((P, D)))
            iota_b = const_p.tile([P, P], bf16)
            nc.sync.dma_start(iota_b[:], t_iob.ap())
            iota_cb = const_p.tile([P, 1], bf16)
            nc.sync.dma_start(iota_cb[:], t_ioc.ap())
            dginv_sb = const_p.tile([P, SLOTS], f32)
            nc.sync.dma_start(dginv_sb[:], t_dginv.ap())
            B_sb = const_p.tile([P, SLOTS, D], bf16)
            nfto_sb = const_p.tile([D + 1, NPAD], bf16)
            nc.sync.dma_start(nfto_sb[:], t_nfto.ap())
            rcnt0 = nc.gpsimd.alloc_register("cnt0")
            rcnt1 = nc.gpsimd.alloc_register("cnt1")

            # ---- phase 1: C = [A | nf] to DRAM; B kept in SBUF
            with contextlib.ExitStack() as pctx:
                pre_p = pctx.enter_context(tc.tile_pool(name="pre", bufs=3))
                pre_ps = pctx.enter_context(
                    tc.tile_pool(name="preps", bufs=2, space="PSUM"))
                NBLK = NF_PAD // P   # 392
                GB = 8
                for g in range(NBLK // GB):
                    c0 = g * GB * P
                    xt = pre_p.tile([D + 1, GB * P], bf16, tag="xt")
                    nc.sync.dma_start(xt[:], t_nft.ap()[:, c0:c0 + GB * P])
                    ps = pre_ps.tile([P, GB, D], f32, tag="ps")
                    for j in range(GB):
                        nc.tensor.matmul(
                            out=ps[:, j, :], lhsT=xt[:, j * P:(j + 1) * P],
                            rhs=w1_sb[:], start=True, stop=True)
                    cb = pre_p.tile([P, GB, 2 * D], bf16, tag="cb")
                    nc.scalar.copy(cb[:, :, :D], ps[:])
                    nc.sync.dma_start(
                        cb[:, :, D:],
                        t_nfb.ap()[c0:c0 + GB * P, :].rearrange(
                            "(b p) d -> p b d", p=P))
                    nc.sync.dma_start(
                        t_C.ap()[c0:c0 + GB * P, :].rearrange(
                            "(b p) d -> p b d", p=P), cb[:])
                # B for own nodes (SLOTS blocks), 8 blocks per group
                for g in range(_ceil_div(SLOTS, GB)):
                    nb = min(GB, SLOTS - g * GB)
                    c0 = g * GB * P
                    ps = pre_ps.tile([P, GB, D], f32, tag="bps")
                    for j in range(nb):
                        nc.tensor.matmul(
                            out=ps[:, j, :],
                            lhsT=nfto_sb[:, c0 + j * P:c0 + (j + 1) * P],
                            rhs=w2_sb[:], start=True, stop=True)
                    nc.scalar.copy(
                        B_sb[:, g * GB:g * GB + nb, :], ps[:, :nb, :])

            # ---- phase 2: per-slot edge processing
            g_p = ctx.enter_context(tc.tile_pool(name="gath", bufs=5))
            sel_p = ctx.enter_context(tc.tile_pool(name="sel", bufs=4))
            e_p = ctx.enter_context(tc.tile_pool(name="edge", bufs=4))
            fin_p = ctx.enter_context(tc.tile_pool(name="fin", bufs=3))
            gbe_ps = ctx.enter_context(
                tc.tile_pool(name="gbeps", bufs=1, space="PSUM"))
            acc_ps = ctx.enter_context(
                tc.tile_pool(name="accps", bufs=2, space="PSUM"))
            fin_ps = ctx.enter_context(
                tc.tile_pool(name="finps", bufs=1, space="PSUM"))

            NCH = _ceil_div(BPS, 8)
            for s in range(SLOTS):
                i0 = g_p.tile([P, B0 * 8], i16, tag="i0")
                nc.sync.dma_start(i0[:], t_is0.ap()[s])
                i1 = g_p.tile([P, B1 * 8], i16, tag="i1")
                nc.sync.dma_start(i1[:], t_is1.ap()[s])
                gC = g_p.tile([P, BPS, 2 * D], bf16, tag="gC")
                nc.gpsimd.dma_gather(
                    out_ap=gC[:, :B0, :], in_ap=t_C.ap()[:HALF, :],
                    idxs_ap=i0[:], num_idxs=B0 * P, num_idxs_reg=B0 * P,
                    elem_size=2 * D, single_packet=False)
                nc.gpsimd.dma_gather(
                    out_ap=gC[:, B0:, :], in_ap=t_C.ap()[HALF:, :],
                    idxs_ap=i1[:], num_idxs=B1 * P, num_idxs_reg=B1 * P,
                    elem_size=2 * D, single_packet=False)
                pm = g_p.tile([P, BPS], bf16, tag="pm")
                nc.sync.dma_start(pm[:], t_pm.ap()[s])
                pmT = g_p.tile([P, BPS * P], bf16, tag="pmT")
                nc.sync.dma_start(
                    pmT[:], t_pmT.ap()[s][None, :].broadcast_to((P, BPS * P)))

                sel = sel_p.tile([P, BPS, P], bf16, tag="sel")
                nc.vector.tensor_tensor(
                    out=sel[:],
                    in0=pm[:].unsqueeze(2).broadcast_to((P, BPS, P)),
                    in1=iota_b[:].unsqueeze(1).broadcast_to((P, BPS, P)),
                    op=_ALU.is_equal)
                selT = sel_p.tile([P, BPS, P], bf16, tag="selT")
                nc.vector.tensor_tensor(
                    out=selT[:],
                    in0=pmT[:].rearrange("p (b e) -> p b e", b=BPS),
                    in1=iota_cb[:].unsqueeze(2).broadcast_to((P, BPS, P)),
                    op=_ALU.is_equal)

                # B-expansion via PE: gBe[e,:] = B_slot[pm_e,:]
                gBe_ps = []
                for ch in range(NCH):
                    bpc = min(8, BPS - ch * 8)
                    psb = gbe_ps.tile([P, bpc * D], f32, tag=f"gbe{ch}")
                    gBe_ps.append(psb)
                    for j in range(bpc):
                        b = ch * 8 + j
                        nc.tensor.matmul(
                            out=psb[:, j * D:(j + 1) * D],
                            lhsT=selT[:, b, :], rhs=B_sb[:, s, :],
                            start=True, stop=True)

                h = e_p.tile([P, BPS, D], bf16, tag="h")
                for ch in range(NCH):
                    bpc = min(8, BPS - ch * 8)
                    sl = slice(ch * 8, ch * 8 + bpc)
                    nc.vector.tensor_add(
                        h[:, sl, :], gC[:, sl, :D],
                        gBe_ps[ch][:].rearrange("p (b d) -> p b d", d=D))
                nc.vector.tensor_scalar_max(h[:], h[:], 0.0)
                hw = e_p.tile([P, BPS, D], bf16, tag="hw")
                nc.vector.tensor_mul(
                    hw[:], h[:],
                    wfc_b[:].unsqueeze(1).broadcast_to((P, BPS, D)))
                lg = e_p.tile([P, BPS], f32, tag="lg")
                nc.vector.tensor_reduce(lg[:], hw[:], axis=_ALT.X, op=_ALU.add)
                # exp(leaky(x)) == max(exp(x), exp(0.01 x)) (monotone)
                we1 = e_p.tile([P, BPS], bf16, tag="we1")
                nc.scalar.activation(we1[:], lg[:], func=_AFT.Exp)
                we2 = e_p.tile([P, BPS], bf16, tag="we2")
                nc.scalar.activation(we2[:], lg[:], func=_AFT.Exp, scale=0.01)
                wt = e_p.tile([P, BPS], bf16, tag="wt")
                nc.vector.tensor_tensor(out=wt[:], in0=we1[:], in1=we2[:],
                                        op=_ALU.max)

                pay = e_p.tile([P, BPS, D + 1], bf16, tag="pay")
                nc.vector.tensor_mul(
                    pay[:, :, :D], gC[:, :, D:],
                    wt[:].unsqueeze(2).broadcast_to((P, BPS, D)))
                nc.scalar.copy(pay[:, :, D], wt[:])

                ps = acc_ps.tile([P, D + 1], f32, tag="ps")
                for b in range(BPS):
                    nc.tensor.matmul(out=ps[:], lhsT=sel[:, b, :],
                                     rhs=pay[:, b, :],
                                     start=(b == 0), stop=(b == BPS - 1))

                # finalize: agg = ps[:, :64]/wsum/deg; out = [nf|agg]@Wn + bn
                den = fin_p.tile([P, 1], f32, tag="den")
                nc.vector.tensor_scalar_max(den[:], ps[:, D:D + 1], 1e-30)
                rec = fin_p.tile([P, 1], f32, tag="rec")
                nc.vector.reciprocal(rec[:], den[:])
                rec2 = fin_p.tile([P, 1], f32, tag="rec2")
                nc.vector.tensor_mul(rec2[:], rec[:], dginv_sb[:, s:s + 1])
                am = fin_p.tile([P, D], f32, tag="am")
                nc.vector.tensor_mul(
                    am[:], ps[:, :D], rec2[:].broadcast_to((P, D)))
                amT_ps = fin_ps.tile([D, P], f32, tag="amT")
                nc.tensor.transpose(out=amT_ps[:], in_=am[:], identity=ident[:])
                amT = fin_p.tile([D, P], bf16, tag="amTs")
                nc.vector.tensor_copy(amT[:], amT_ps[:])
                o_ps = fin_ps.tile([P, D], f32, tag="ops")
                nc.tensor.matmul(out=o_ps[:],
                                 lhsT=nfto_sb[:D, s * P:(s + 1) * P],
                                 rhs=wn1_sb[:], start=True, stop=False)
                nc.tensor.matmul(out=o_ps[:], lhsT=amT[:], rhs=wn2_sb[:],
                                 start=False, stop=True)
                o_sb = fin_p.tile([P, D], f32, tag="osb")
                nc.vector.tensor_add(o_sb[:], o_ps[:], bn_b[:])
                nc.sync.dma_start(t_out.ap()[s * P:(s + 1) * P, :], o_sb[:])

    nc.compile()
    return nc


_PROGRAM_CACHE = {}


def _get_program(cfg_key, cfg):
    if cfg_key not in _PROGRAM_CACHE:
        _PROGRAM_CACHE[cfg_key] = _build_program(cfg)
    return _PROGRAM_CACHE[cfg_key]


def _prep(nf, src, dst, W_attn, b_attn, w_fc, W_node, b_node, n_cores=8):
    from ml_dtypes import bfloat16

    N = nf.shape[0]
    NPC = N // n_cores
    NPAD = _ceil_div(NPC, P) * P
    SLOTS = NPAD // P
    HALF = _ceil_div(_ceil_div(N, 2), P) * P
    NF_PAD = 2 * HALF
    assert HALF <= 32768 and N - HALF <= HALF

    src = np.asarray(src).astype(np.int64)
    dst = np.asarray(dst).astype(np.int64)
    nf = np.asarray(nf, dtype=np.float32)

    owner = dst // NPC
    dst_local = dst - owner * NPC
    g = dst_local >> 7
    pmod = dst_local & 127
    half = (src >= HALF).astype(np.int64)
    src_local = src - half * HALF

    key = (owner * SLOTS + g) * 2 + half
    order = np.argsort(key, kind="stable")
    skey = key[order]
    nkeys = n_cores * SLOTS * 2
    bounds = np.searchsorted(skey, np.arange(nkeys + 1))
    cnt = np.diff(bounds).reshape(n_cores, SLOTS, 2)
    B0 = max(1, int(_ceil_div(cnt[:, :, 0].max(), P)))
    B1 = max(1, int(_ceil_div(cnt[:, :, 1].max(), P)))
    BPS = B0 + B1

    # -1 padding: trailing -1 idxs are skipped by the gather ucode; pm = -1
    # makes sel/selT columns all-zero so padded lanes contribute nothing.
    p_src = np.zeros((n_cores, SLOTS, BPS * P), np.int16)
    p_pm = np.full((n_cores, SLOTS, BPS * P), -1.0, np.float32)
    p_cnt = np.zeros((n_cores, SLOTS, 2), np.uint32)
    for c in range(n_cores):
        for s in range(SLOTS):
            for h in (0, 1):
                k = (c * SLOTS + s) * 2 + h
                seg = order[bounds[k]:bounds[k + 1]]
                off = 0 if h == 0 else B0 * P
                n = len(seg)
                p_src[c, s, off:off + n] = src_local[seg]
                p_pm[c, s, off:off + n] = pmod[seg]
                p_cnt[c, s, h] = n

    deg = np.bincount(dst, minlength=N).astype(np.float32)

    cfg = {"NF_PAD": NF_PAD, "HALF": HALF, "NPAD": NPAD, "B0": B0, "B1": B1,
           "OWN0_PER_CORE": 0}

    nf_pad = np.zeros((NF_PAD, D), np.float32)
    nf_pad[:N] = nf
    nfb = nf_pad.astype(bfloat16)
    nft = np.ones((D + 1, NF_PAD), np.float32)
    nft[:D] = nf_pad.T
    nft = nft.astype(bfloat16)
    w1a = np.concatenate([W_attn[:D], np.asarray(b_attn)[None, :]], axis=0)
    w2a = np.concatenate([W_attn[D:], np.zeros((1, D), np.float32)], axis=0)
    iob = np.broadcast_to(np.arange(P, dtype=np.float32), (P, P))
    ioc = np.arange(P, dtype=np.float32)[:, None]

    in_maps = []
    for c in range(n_cores):
        is0 = _wrap16(p_src[c, :, :B0 * P])
        is1 = _wrap16(p_src[c, :, B0 * P:])
        pm = np.ascontiguousarray(
            p_pm[c].reshape(SLOTS, BPS, P).transpose(0, 2, 1)).astype(bfloat16)
        pmT = np.ascontiguousarray(p_pm[c]).astype(bfloat16)
        own0 = c * NPC
        nfto = np.ones((D + 1, NPAD), np.float32)
        nfto[:D, :] = 0.0
        nfto[:D, :min(NPAD, NF_PAD - own0)] = \
            nf_pad.T[:, own0:own0 + NPAD]
        degc = np.ones(NPAD, np.float32)
        degc[:NPC] = np.maximum(deg[own0:own0 + NPC], 1.0)
        dginv = np.ascontiguousarray((1.0 / degc).reshape(SLOTS, P).T)
        in_maps.append({
            "nfb": nfb, "nft": nft,
            "nfto": np.ascontiguousarray(nfto).astype(bfloat16),
            "w1": np.ascontiguousarray(w1a).astype(bfloat16),
            "w2": np.ascontiguousarray(w2a).astype(bfloat16),
            "wfc": np.asarray(w_fc).astype(bfloat16),
            "wn1": np.ascontiguousarray(W_node[:D]).astype(bfloat16),
            "wn2": np.ascontiguousarray(W_node[D:]).astype(bfloat16),
            "bn": np.asarray(b_node, np.float32),
            "iob": np.ascontiguousarray(iob).astype(bfloat16),
            "ioc": np.ascontiguousarray(ioc).astype(bfloat16),
            "is0": is0, "is1": is1, "pm": pm, "pmT": pmT,
            "dginv": dginv,
            "cnt": np.ascontiguousarray(
                p_cnt[c].reshape(1, SLOTS * 2)),
        })
    return cfg, in_maps, NPC


def kernel(nf, src, dst, W_attn, b_attn, w_fc, W_node, b_node):
    import concourse.bass_utils as bass_utils

    nf = np.asarray(nf)
    cfg, in_maps, NPC = _prep(nf, src, dst, W_attn, b_attn, w_fc,
                              W_node, b_node)
    cfg_key = tuple(sorted(cfg.items()))
    nc = _get_program(cfg_key, cfg)
    res = bass_utils.run_bass_kernel_spmd(nc, in_maps,
                                          core_ids=list(range(8)))
    out = np.concatenate([res.results[c]["out"][:NPC] for c in range(8)],
                         axis=0)
    return out.astype(np.float32)

